# revision 15
# baseline (speedup 1.0000x reference)
"""HNetv1 Trainium2 Bass kernel (v3, fp8 weights).

Strategy (8 NeuronCores):
  - Every core computes the l2-normalized correlation for ALL 64 batches
    (redundantly) -> X[64, 20736] as lhsT tiles X_a/X_b in float8e4.
  - Layer 1 (x @ w1) is column-split 8 ways.  Each core's w1 slice
    [20736, 648] is host-scaled by WS=2048 and cast to float8_e4m3
    ([128, 162, 648], 13.4 MB, resident in SBUF; the stream is dispatched
    mid-corr on the scalar HWDGE ring so the corr front-half is free of
    SBUF/DMA contention).  L1 runs as mixed-precision matmuls (lhsT = X
    bf16, rhs = w1 fp8): 162 k-tiles x 2 column halves.  The 1/WS dequant
    scale is folded into the ReLU activation scale.  Final output error
    ~0.5% (validated in numpy): the tanh in L3 attenuates the fp8
    quantization error ~8x.
  - h1 [64,648] is transposed on-chip (PE) to h1T [128,6,64]; the L2
    partial is computed in transposed orientation against the fp8 w2
    row-slice (o padded to 1408 = 11*128), with b2*WS/8 added by every
    core; a single AllReduce(add) of [128,11,64] bf16 then yields full
    pre-activation h2^T * WS.  ReLU(scale=1/WS), then L3 (transposed
    h3T, tanh) and L4 run redundantly on every core.

Layouts (host-prepped):
  x1r/x2r: [C=128, N=64, HW=144] bf16.
  w1f: [128, 162, 648] float8e4 (k-tile t<144: w1 rows t*144+p; t=144+bo,
       p=16*bi+r: row 144*k+128+r, k=8*bo+bi -- matches X_b regrouping).
  w2p: [128, 6, 1408] float8e4 (row 128*t+p of the core's 648 w2 rows)
  w3p: [128, 11, 324] bf16; w4p: [128, 3, 8] bf16.
"""

import os
import numpy as np
import ml_dtypes

N, C, S = 64, 128, 12
HW = S * S            # 144
RIN = S ** 4          # 20736
NCORES = 8
COLS1 = 5184 // NCORES   # 648
HALF1 = COLS1 // 2       # 324
NT1 = 162                # L1 k-tiles (144 main + 18 residue)
WS = 2048.0              # fp8 weight scale for w1/w2
O2 = 1408                # padded w2 out cols (11*128)
NOC2 = 11
NKT2 = 6                 # 768/128
NKT3 = 11                # 1408/128
NOC3 = 3                 # 384/128 (324 padded)

_CACHE = {}

LAST_RESULT = None  # BassKernelResults from the most recent run (for test.py)


def _bf16(a):
    return np.asarray(a, dtype=np.float32).astype(ml_dtypes.bfloat16)


def _fp8(a):
    return np.asarray(a, dtype=np.float32).astype(ml_dtypes.float8_e4m3)


def _build_nc():
    import concourse.bacc as bacc
    import concourse.tile as tile
    import concourse.mybir as mybir
    from concourse.masks import make_identity

    dt = mybir.dt
    AF = mybir.ActivationFunctionType
    ALU = mybir.AluOpType
    DR = mybir.MatmulPerfMode.DoubleRow

    nc = bacc.Bacc("TRN2", target_bir_lowering=False, debug=False,
                   num_devices=NCORES)

    x1r_d = nc.dram_tensor("x1r", [C, N, HW], dt.bfloat16, kind="ExternalInput")
    x2r_d = nc.dram_tensor("x2r", [C, N, HW], dt.bfloat16, kind="ExternalInput")
    w1f_d = nc.dram_tensor("w1f", [128, NT1, COLS1], dt.float8e4, kind="ExternalInput")
    b1lo_d = nc.dram_tensor("b1lo", [1, HALF1], dt.bfloat16, kind="ExternalInput")
    b1hi_d = nc.dram_tensor("b1hi", [1, HALF1], dt.bfloat16, kind="ExternalInput")
    w2p_d = nc.dram_tensor("w2p", [128, NKT2, O2], dt.float8e4, kind="ExternalInput")
    b2_d = nc.dram_tensor("b2r", [1, O2], dt.bfloat16, kind="ExternalInput")
    w3p_d = nc.dram_tensor("w3p", [128, NKT3, 324], dt.bfloat16, kind="ExternalInput")
    b3_d = nc.dram_tensor("b3r", [1, 324], dt.bfloat16, kind="ExternalInput")
    w4p_d = nc.dram_tensor("w4p", [128, NOC3, 8], dt.bfloat16, kind="ExternalInput")
    b4_d = nc.dram_tensor("b4r", [1, 8], dt.bfloat16, kind="ExternalInput")
    out_d = nc.dram_tensor("out", [N, 8], dt.float32, kind="ExternalOutput")

    rg = [list(range(NCORES))]

    with tile.TileContext(nc) as tc:
        with tc.tile_pool(name="persist", bufs=1) as persist, \
             tc.tile_pool(name="w1pool", bufs=1) as w1pool, \
             tc.tile_pool(name="dramp", bufs=1, space="DRAM") as dramp, \
             tc.tile_pool(name="pl1", bufs=1, space="PSUM") as pl1:
            # ---- persistent tiles / constants ----
            ones128 = persist.tile([128, 128], dt.bfloat16)
            nc.vector.memset(ones128[:], 1.0)
            onesrow = persist.tile([1, N], dt.bfloat16)
            nc.vector.memset(onesrow[:], 1.0)
            ident = persist.tile([128, 128], dt.bfloat16)
            make_identity(nc, ident[:])

            X_a = persist.tile([128, HW, N], dt.bfloat16)     # [ij, k, n]
            X_b = persist.tile([128, 18, N], dt.bfloat16)     # [16*bi+r, bo, n]

            h1sb = persist.tile([64, 768], dt.bfloat16)
            nc.vector.memset(h1sb[:], 0.0)
            h1T = persist.tile([128, NKT2, N], dt.bfloat16)
            h2T = persist.tile([128, NKT3, N], dt.bfloat16)
            h2a = persist.tile([128, NKT3, N], dt.bfloat16)
            h3T = persist.tile([128, NOC3, N], dt.bfloat16)
            outsb = persist.tile([64, 8], dt.float32)

            # internal DRAM for the single AllReduce
            ar_in = dramp.tile([128, NKT3, N], dt.bfloat16)
            ar_out = dramp.tile([128, NKT3, N], dt.bfloat16, addr_space="Shared")

            # ---- w1 resident load (9 big slices, issued immediately) ----
            # On the scalar-engine HWDGE ring so the x1/x2 correlation loads
            # (sync ring) are not FIFO-queued behind 13.4 MB of weights.
            w1sb = w1pool.tile([128, NT1, COLS1], dt.float8e4)

            # L1 psums (2 banks, live through corr as well)
            ps0 = pl1.tile([64, HALF1], dt.float32, name="ps0")
            ps1 = pl1.tile([64, HALF1], dt.float32, name="ps1")

            # ---------------- corr phase ----------------
            G = 4
            NGRP = N // G
            with tc.tile_pool(name="cx", bufs=4) as cx, \
                 tc.tile_pool(name="csq", bufs=3) as csq, \
                 tc.tile_pool(name="cs", bufs=4) as cs, \
                 tc.tile_pool(name="cr", bufs=4) as cr, \
                 tc.tile_pool(name="cxs", bufs=3) as cxs, \
                 tc.tile_pool(name="xbt", bufs=1) as xbt, \
                 tc.tile_pool(name="pssq", bufs=1, space="PSUM") as pssq, \
                 tc.tile_pool(name="pca", bufs=3, space="PSUM") as pca, \
                 tc.tile_pool(name="pcb", bufs=1, space="PSUM") as pcb:
                X_btmp = xbt.tile([16, HW, N], dt.bfloat16)   # [r, k, n]
                for g in range(NGRP):
                    if g == 6:
                        # start the 13.4 MB w1 stream mid-corr (scalar HWDGE
                        # ring): the corr front-half runs contention-free and
                        # the stream still completes before L1 consumes it
                        for sl in range(9):
                            t0 = 18 * sl
                            nc.scalar.dma_start(w1sb[:, t0:t0 + 18, :],
                                                w1f_d[:, t0:t0 + 18, :])
                    n0 = G * g
                    x1t = cx.tile([C, G, HW], dt.bfloat16, tag="x1t")
                    nc.sync.dma_start(x1t[:], x1r_d[:, n0:n0 + G, :])
                    x2t = cx.tile([C, G, HW], dt.bfloat16, tag="x2t")
                    nc.sync.dma_start(x2t[:], x2r_d[:, n0:n0 + G, :])

                    for p in range(2):
                        bsl = slice(2 * p, 2 * p + 2)
                        # squares (pure bf16 TT -> DVE/pool fast modes)
                        sq1 = csq.tile([C, 2, HW], dt.bfloat16, tag="sq1")
                        nc.vector.tensor_tensor(sq1[:], x1t[:, bsl, :],
                                                x1t[:, bsl, :], ALU.mult)
                        sq2 = csq.tile([C, 2, HW], dt.bfloat16, tag="sq2")
                        nc.gpsimd.tensor_tensor(sq2[:], x2t[:, bsl, :],
                                                x2t[:, bsl, :], ALU.mult)
                        q1 = pssq.tile([128, 512], dt.float32, tag="q1")
                        nc.tensor.matmul(q1[:, 0:2 * HW], ones128[:],
                                         sq1[:].rearrange("p a k -> p (a k)"),
                                         start=True, stop=True)
                        q2 = pssq.tile([128, 512], dt.float32, tag="q2")
                        nc.tensor.matmul(q2[:, 0:2 * HW], ones128[:],
                                         sq2[:].rearrange("p a k -> p (a k)"),
                                         start=True, stop=True)
                        # norms (scalar, bf16 out) and reciprocals (vector)
                        s1 = cs.tile([128, 2 * HW], dt.float32, tag="s1")
                        nc.scalar.activation(s1[:], q1[:, 0:2 * HW], AF.Sqrt)
                        s2 = cs.tile([128, 2 * HW], dt.float32, tag="s2")
                        nc.scalar.activation(s2[:], q2[:, 0:2 * HW], AF.Sqrt)
                        r1 = cr.tile([128, 2 * HW], dt.float32, tag="r1")
                        nc.vector.reciprocal_approx_fast(r1[:], s1[:])
                        r2 = cr.tile([128, 2 * HW], dt.float32, tag="r2")
                        nc.vector.reciprocal_approx_fast(r2[:], s2[:])
                        # normalized inputs (pure bf16 TT)
                        x1s = cxs.tile([C, 2, HW], dt.bfloat16, tag="x1s")
                        nc.gpsimd.tensor_tensor(
                            x1s[:].rearrange("p a k -> p (a k)"),
                            x1t[:, bsl, :].rearrange("p a k -> p (a k)"),
                            r1[:], ALU.mult)
                        x2s = cxs.tile([C, 2, HW], dt.bfloat16, tag="x2s")
                        nc.vector.tensor_tensor(
                            x2s[:].rearrange("p a k -> p (a k)"),
                            x2t[:, bsl, :].rearrange("p a k -> p (a k)"),
                            r2[:], ALU.mult)
                        # correlation matmuls
                        ca = pca.tile([128, 2, HW], dt.float32, tag="ca")
                        cb = pcb.tile([16, 2, HW], dt.float32, tag="cb")
                        for bb in range(2):
                            nc.tensor.matmul(ca[:, bb, :], x1s[:, bb, 0:128],
                                             x2s[:, bb, :], start=True, stop=True)
                            nc.tensor.matmul(cb[:, bb, :], x1s[:, bb, 128:HW],
                                             x2s[:, bb, :], start=True, stop=True)
                        # psum -> X (cast to bf16)
                        m0 = n0 + 2 * p
                        xa_out = X_a[:, :, m0:m0 + 2].rearrange("p k b -> p b k")
                        if p == 0:
                            nc.scalar.copy(xa_out, ca[:])
                        else:
                            nc.vector.tensor_copy(xa_out, ca[:])
                        xb_out = X_btmp[:, :, m0:m0 + 2].rearrange("r k b -> r b k")
                        if p == 0:
                            nc.vector.tensor_copy(xb_out, cb[:])
                        else:
                            nc.scalar.copy(xb_out, cb[:])

                # regroup residue: X_b[16*bi+r, bo, n] = X_btmp[r, 8*bo+bi, n]
                xbtv = X_btmp[:].rearrange("r (bo bi) n -> r bo bi n", bi=8)
                for bi in range(8):
                    nc.sync.dma_start(X_b[16 * bi:16 * bi + 16, :, :],
                                      xbtv[:, :, bi, :])

            # ---- tail weights (loaded during L1; reuses corr SBUF) ----
            with tc.tile_pool(name="tw", bufs=1) as tw:
                w2sb = tw.tile([128, NKT2, O2], dt.float8e4)
                nc.sync.dma_start(w2sb[:], w2p_d[:, :, :])
                w3sb = tw.tile([128, NKT3, 324], dt.bfloat16)
                nc.sync.dma_start(w3sb[:], w3p_d[:, :, :])
                w4sb = tw.tile([128, NOC3, 8], dt.bfloat16)
                nc.sync.dma_start(w4sb[:], w4p_d[:, :, :])
                b1lo = tw.tile([1, HALF1], dt.bfloat16)
                nc.sync.dma_start(b1lo[:], b1lo_d[:, :])
                b1hi = tw.tile([1, HALF1], dt.bfloat16)
                nc.sync.dma_start(b1hi[:], b1hi_d[:, :])
                b2sb = tw.tile([1, O2], dt.bfloat16)
                nc.sync.dma_start(b2sb[:], b2_d[:, :])
                b3sb = tw.tile([1, 324], dt.bfloat16)
                nc.sync.dma_start(b3sb[:], b3_d[:, :])
                b4sb = tw.tile([1, 8], dt.bfloat16)
                nc.sync.dma_start(b4sb[:], b4_d[:, :])

                # ------- L1: mixed bf16(lhsT) x fp8(rhs) matmuls, 162 k-tiles
                for t in range(NT1):
                    lhsT = X_a[:, t, :] if t < HW else X_b[:, t - HW, :]
                    nc.tensor.matmul(ps0[:], lhsT, w1sb[:, t, 0:HALF1],
                                     start=(t == 0), stop=False)
                    nc.tensor.matmul(ps1[:], lhsT, w1sb[:, t, HALF1:COLS1],
                                     start=(t == 0), stop=False)

                # bias (b1*WS) and accumulation stop
                nc.tensor.matmul(ps0[:], onesrow[:], b1lo[:],
                                 start=False, stop=True)
                nc.tensor.matmul(ps1[:], onesrow[:], b1hi[:],
                                 start=False, stop=True)

                # h1 = relu(acc/WS + b1)
                nc.scalar.activation(h1sb[:, 0:HALF1], ps0[:], AF.Relu,
                                     scale=float(1.0 / WS))
                nc.scalar.activation(h1sb[:, HALF1:COLS1], ps1[:], AF.Relu,
                                     scale=float(1.0 / WS))

                # ---- transpose h1 -> h1T [128, 6, 64] ----
                with tc.tile_pool(name="ptp", bufs=2, space="PSUM") as ptp:
                    for t in range(NKT2):
                        tp = ptp.tile([128, 64], dt.bfloat16, tag="tp")
                        nc.tensor.transpose(tp[:], h1sb[:, 128 * t:128 * (t + 1)],
                                            ident[0:64, 0:64])
                        nc.vector.tensor_copy(h1T[:, t, :], tp[:])

                # ---- L2 partial, transposed: ps[o, n] over o-chunks ----
                with tc.tile_pool(name="pl2", bufs=1, space="PSUM") as pl2:
                    psA = pl2.tile([128, 6, N], dt.float32, name="psA")
                    psB = pl2.tile([128, 5, N], dt.float32, name="psB")

                    def l2ps(oc):
                        return psA[:, oc, :] if oc < 6 else psB[:, oc - 6, :]

                    for oc in range(NOC2):
                        for t in range(NKT2):
                            nc.tensor.matmul(
                                l2ps(oc), w2sb[:, t, 128 * oc:128 * (oc + 1)],
                                h1T[:, t, :], start=(t == 0), stop=False)
                        # + b2*WS/8 (summed across cores by the AllReduce)
                        nc.tensor.matmul(
                            l2ps(oc),
                            b2sb[:, 128 * oc:128 * (oc + 1)], onesrow[:],
                            start=False, stop=True)
                    # psum -> bf16 staging, out to DRAM, AllReduce
                    arst = tw.tile([128, NKT3, N], dt.bfloat16)
                    nc.vector.tensor_copy(arst[:, 0:6, :], psA[:])
                    nc.vector.tensor_copy(arst[:, 6:11, :], psB[:])
                    nc.sync.dma_start(ar_in[:], arst[:])
                nc.gpsimd.collective_compute(
                    "AllReduce", mybir.AluOpType.add, replica_groups=rg,
                    ins=[ar_in[:]], outs=[ar_out[:]])
                nc.sync.dma_start(h2T[:], ar_out[:])

                # ---- ReLU (undo WS), L3 (transposed out), tanh ----
                nc.scalar.activation(
                    h2a[:].rearrange("p t n -> p (t n)"),
                    h2T[:].rearrange("p t n -> p (t n)"), AF.Relu,
                    scale=float(1.0 / WS))

                with tc.tile_pool(name="pl3", bufs=1, space="PSUM") as pl3, \
                     tc.tile_pool(name="pl4", bufs=1, space="PSUM") as pl4:
                    ps3 = pl3.tile([128, NOC3, N], dt.float32, name="ps3")
                    for c3 in range(NOC3):
                        cw = min(128, 324 - 128 * c3)
                        for t in range(NKT3):
                            nc.tensor.matmul(
                                ps3[0:cw, c3, :],
                                w3sb[:, t, 128 * c3:128 * c3 + cw],
                                h2a[:, t, :], start=(t == 0), stop=False)
                        nc.tensor.matmul(
                            ps3[0:cw, c3, :],
                            b3sb[:, 128 * c3:128 * c3 + cw], onesrow[:],
                            start=False, stop=True)
                        nc.scalar.activation(h3T[0:cw, c3, :], ps3[0:cw, c3, :],
                                             AF.Tanh)

                    # ---- L4 ----
                    ps4 = pl4.tile([64, 8], dt.float32, name="ps4")
                    for c3 in range(NOC3):
                        cw = min(128, 324 - 128 * c3)
                        nc.tensor.matmul(ps4[:], h3T[0:cw, c3, :],
                                         w4sb[0:cw, c3, :],
                                         start=(c3 == 0), stop=False)
                    nc.tensor.matmul(ps4[:], onesrow[:], b4sb[:],
                                     start=False, stop=True)
                    nc.vector.tensor_copy(outsb[:], ps4[:])
                    nc.sync.dma_start(out_d[:, :], outsb[:])

    nc.compile()
    return nc


def _pack_w1(w1):
    """Per-core fp8 k-tile packs [128, 162, 648] (scaled by WS)."""
    w1 = np.asarray(w1, np.float32) * WS
    packs = []
    for core in range(NCORES):
        wc = w1[:, COLS1 * core:COLS1 * (core + 1)].reshape(HW, HW, COLS1)
        main = wc[:, 0:128, :]                      # [144, 128, 648]
        res = wc[:, 128:HW, :].reshape(18, 8, 16, COLS1).reshape(18, 128, COLS1)
        m = np.concatenate([main, res], axis=0)     # [162, 128, 648]
        packs.append(_fp8(np.ascontiguousarray(m.transpose(1, 0, 2))))
    return packs


def _prep_inputs(x1, x2, w1, b1, w2, b2, w3, b3, w4, b4):
    """Host-side shard/permute/cast. Returns per-core input maps."""
    x1f = np.asarray(x1, np.float32).reshape(N, C, HW)
    x2f = np.asarray(x2, np.float32).reshape(N, C, HW)
    x1r = _bf16(np.ascontiguousarray(x1f.transpose(1, 0, 2)))
    x2r = _bf16(np.ascontiguousarray(x2f.transpose(1, 0, 2)))
    b1 = np.asarray(b1, np.float32)
    w2 = np.asarray(w2, np.float32)
    b2 = np.asarray(b2, np.float32)
    w3 = np.asarray(w3, np.float32)
    b3 = np.asarray(b3, np.float32)
    w4 = np.asarray(w4, np.float32)
    b4 = np.asarray(b4, np.float32)

    w1packs = _pack_w1(w1)

    w3pad = np.zeros((NKT3 * 128, 324), np.float32)
    w3pad[0:1296] = w3
    w3p = _bf16(np.ascontiguousarray(
        w3pad.reshape(NKT3, 128, 324).transpose(1, 0, 2)))
    w4pad = np.zeros((NOC3 * 128, 8), np.float32)
    w4pad[0:324] = w4
    w4p = _bf16(np.ascontiguousarray(
        w4pad.reshape(NOC3, 128, 8).transpose(1, 0, 2)))
    b3r = _bf16(b3).reshape(1, 324)
    b4r = _bf16(b4).reshape(1, 8)
    b2pad = np.zeros((O2,), np.float32)
    b2pad[0:1296] = b2 * WS / NCORES
    b2r = _bf16(b2pad).reshape(1, O2)

    in_maps = []
    for core in range(NCORES):
        # w2 row-slice for this core, scaled by WS, padded -> [128, 6, 1408]
        w2pad = np.zeros((NKT2 * 128, O2), np.float32)
        w2pad[0:COLS1, 0:1296] = w2[COLS1 * core:COLS1 * (core + 1)] * WS
        w2p = _fp8(np.ascontiguousarray(
            w2pad.reshape(NKT2, 128, O2).transpose(1, 0, 2)))
        b1c = b1[COLS1 * core:COLS1 * (core + 1)] * WS
        in_maps.append({
            "x1r": x1r, "x2r": x2r,
            "w1f": w1packs[core],
            "b1lo": _bf16(b1c[0:HALF1]).reshape(1, HALF1),
            "b1hi": _bf16(b1c[HALF1:COLS1]).reshape(1, HALF1),
            "w2p": w2p, "b2r": b2r,
            "w3p": w3p, "b3r": b3r,
            "w4p": w4p, "b4r": b4r,
        })
    return in_maps


def kernel(x1, x2, w1, b1, w2, b2, w3, b3, w4, b4):
    global LAST_RESULT

    in_maps = _prep_inputs(x1, x2, w1, b1, w2, b2, w3, b3, w4, b4)
    if "nc" not in _CACHE:
        _CACHE["nc"] = _build_nc()
    nc = _CACHE["nc"]

    if os.environ.get("HNET_SIM", "0") == "1":
        from concourse.bass_interp import MultiCoreSim
        sim = MultiCoreSim(nc, num_cores=NCORES)
        cores = list(sim.cores.values())
        for cid, core_sim in enumerate(cores):
            for k, v in in_maps[cid].items():
                core_sim.tensor(k)[:] = v
        sim.simulate(check_with_hw=False)
        H = np.asarray(cores[0].tensor("out"), np.float32)
    else:
        from concourse.bass_utils import run_bass_kernel_spmd
        trace = bool(int(os.environ.get("HNET_TRACE", "0")))
        res = run_bass_kernel_spmd(nc, in_maps, core_ids=list(range(NCORES)),
                                   trace=trace)
        LAST_RESULT = res
        H = np.asarray(res.results[0]["out"], np.float32)

    ones = np.ones((N, 1), np.float32)
    return np.concatenate([H, ones], axis=1).reshape(N, 3, 3)
